# revision 10
# baseline (speedup 1.0000x reference)
"""Trainium2 Bass kernel for nn_AttentionBlock (GroupNorm + 8-head attention).

Sharding: 8 cores = 4 batches x 2 head-groups (4 heads per core).
Each core computes GroupNorm (duplicated within a batch pair), the QKV
projection for its heads, attention, and a partial output projection.
The host sums the two partials per batch and adds bias + residual.

v6 design notes (vs v5: interleaved head pairs, per-tile GN, no swaps):
  - ACT floor per core: 32 exp ACTIVATEs of [128, 1024] ~= 35.6 us.
    Everything else is scheduled to hide under the exp stream.
  - GroupNorm groups (16 ch) never cross 128-channel tiles, so stats /
    alpha / beta / xn are per-tile pipelines chasing the x DMA (3 queues:
    sync, scalar, gpsimd).  Stats: DVE bn_stats for tiles 0,2,3; ACT
    Copy/Square+accum for tile 1.  rstd = exp(-0.5*ln(var+eps)) on ACT so
    the whole kernel uses ONE table set (natural_log_exp_and_others).
  - Heads of a pair are interleaved in the stream: head 2p at PE rows
    0:64, head 2p+1 at rows 64:128 -> adjacent score matmuls overlap in
    distinct row groups with NO partition-swapped copies (v5's 1MB of
    SBUF->SBUF swap DMA is gone).
  - PSUM: tag "sc" 2x[128,1024] (4 banks) carries qk-pair0 psums, score
    tiles, qk23 chunks (paired insertions to keep rotation parity), dbc
    drain broadcasts, and tail yps tiles.  Tag "o" 4x one-bank carries
    GN gps/mvx, v-proj pv, and the 4 A@V accumulators of the live pair.
  - Drains are v5's zero-DMA normalize (ones-row K=1 PE broadcast of the
    denominator row + reciprocal_approx_fast + TT mul), one [64,1024]
    recip/mul per head.
  - Tail: h2 staggered one slot before h3; out-proj per token tile with
    resT0 matmuls pre-run where banks free early; y DMA over 3 queues.
"""

import ml_dtypes
import numpy as np

import concourse.bass as bass
import concourse.bacc as bacc
import concourse.tile as tile
from concourse import mybir
from concourse.bass_utils import run_bass_kernel_spmd

FP32 = mybir.dt.float32
BF16 = mybir.dt.bfloat16

B, HH, WW, C = 4, 32, 32, 512
N = HH * WW              # 1024 tokens
N_HEADS = 8
HD = C // N_HEADS        # 64
N_GROUPS = 32
GS = C // N_GROUPS       # 16 channels per group
GN_EPS = 1e-6
SCALE = C ** -0.5
NHC = 4                  # heads per core
P = 128
CT = C // P              # 4 channel tiles
TT = N // P              # 8 token tiles
NCORES = 8
AF = mybir.ActivationFunctionType


def _mm(nc, out, lhsT, rhs, start, stop, tile_position=None):
    nc.tensor.matmul(out, lhsT, rhs, start=start, stop=stop,
                     tile_position=tile_position)


def _build_group_mats(nc, consts):
    """G [128, 8] with G[c,g] = (c//16 == g)/16, and GT [8, 128] = 1s mask."""
    G = consts.tile([P, 8], FP32, name="G")
    nc.gpsimd.memset(G, 1.0 / GS)
    nc.gpsimd.affine_select(out=G, in_=G, compare_op=mybir.AluOpType.is_ge,
                            fill=0.0, base=0, pattern=[[-GS, 8]],
                            channel_multiplier=1)
    nc.gpsimd.affine_select(out=G, in_=G, compare_op=mybir.AluOpType.is_ge,
                            fill=0.0, base=GS - 1, pattern=[[GS, 8]],
                            channel_multiplier=-1)
    GT = consts.tile([8, P], FP32, name="GT")
    nc.gpsimd.memset(GT, 1.0)
    nc.gpsimd.affine_select(out=GT, in_=GT, compare_op=mybir.AluOpType.is_ge,
                            fill=0.0, base=0, pattern=[[1, P]],
                            channel_multiplier=-GS)
    nc.gpsimd.affine_select(out=GT, in_=GT, compare_op=mybir.AluOpType.is_ge,
                            fill=0.0, base=GS - 1, pattern=[[-1, P]],
                            channel_multiplier=GS)
    return G, GT


def build_program(compile=True):
    nc = bacc.Bacc()
    xT = nc.dram_tensor("xT", [C, N], BF16, kind="ExternalInput").ap()
    gsb2 = nc.dram_tensor("gsb2", [2, C], FP32, kind="ExternalInput").ap()
    wqkA = nc.dram_tensor("wqkA", [C, 256], BF16, kind="ExternalInput").ap()
    wqkB = nc.dram_tensor("wqkB", [C, 256], BF16, kind="ExternalInput").ap()
    wv = nc.dram_tensor("wv", [C, NHC * HD], BF16, kind="ExternalInput").ap()
    wo = nc.dram_tensor("wo", [NHC * HD, C], BF16, kind="ExternalInput").ap()
    y = nc.dram_tensor("y", [N, C], BF16, kind="ExternalOutput").ap()

    with tile.TileContext(nc) as tc:
        with (
            tc.tile_pool(name="consts", bufs=1) as consts,
            tc.tile_pool(name="xts", bufs=1) as xts,
            tc.tile_pool(name="wpool", bufs=1) as wpool,
            tc.tile_pool(name="qk", bufs=1) as qkpool,
            tc.tile_pool(name="vp", bufs=1) as vpool,
            tc.tile_pool(name="ep", bufs=4) as epool,
            tc.tile_pool(name="osb", bufs=2) as osbpool,
            tc.tile_pool(name="small", bufs=1) as small,
            tc.tile_pool(name="res", bufs=1) as respool,
            tc.tile_pool(name="yp", bufs=1) as ypool,
            tc.tile_pool(name="ps", bufs=1, space="PSUM") as ps,
        ):
            # PSUM tags: "sc" 2x[128,1024] = 4 banks, "o" 4x[128,512] = 4.
            def ps_sc(name, shape=None):
                return ps.tile(shape or [P, N], FP32, name=name, tag="sc",
                               bufs=2)

            def ps_o(name, shape=None):
                return ps.tile(shape or [P, 512], FP32, name=name, tag="o",
                               bufs=4)

            # ---------------- input DMAs, 3 queues ------------------------
            xt = [xts.tile([P, N], BF16, name=f"xt{k}") for k in range(CT)]
            gsb_sb = consts.tile([P, 2, CT], FP32, name="gsb_sb")
            wqkA_sb = wpool.tile([P, CT, 256], BF16, name="wqkA_sb")
            wqkB_sb = wpool.tile([P, CT, 256], BF16, name="wqkB_sb")
            wv_sb = wpool.tile([P, CT, NHC * HD], BF16, name="wv_sb")
            wo_sb = wpool.tile([P, 2, C], BF16, name="wo_sb")

            def _ap(t, ap):
                return bass.AP(tensor=t.tensor, offset=t.offset, ap=ap)

            # sync: x0, x3h0, gsb2, wqkA, wo
            nc.sync.dma_start(out=xt[0], in_=xT[0:P, :])
            nc.sync.dma_start(out=xt[3][:, 0:512], in_=xT[3 * P:4 * P, 0:512])
            nc.sync.dma_start(out=gsb_sb,
                              in_=_ap(gsb2, [[1, P], [C, 2], [P, CT]]))
            nc.sync.dma_start(out=wqkA_sb,
                              in_=_ap(wqkA, [[256, P], [P * 256, CT],
                                             [1, 256]]))
            nc.sync.dma_start(out=wo_sb,
                              in_=_ap(wo, [[C, P], [P * C, 2], [1, C]]))
            # scalar: x1, x3h1, wv, wqkB
            nc.scalar.dma_start(out=xt[1], in_=xT[P:2 * P, :])
            nc.scalar.dma_start(out=xt[3][:, 512:1024],
                                in_=xT[3 * P:4 * P, 512:1024])
            nc.scalar.dma_start(out=wv_sb,
                                in_=_ap(wv, [[256, P], [P * 256, CT],
                                             [1, 256]]))
            nc.scalar.dma_start(out=wqkB_sb,
                                in_=_ap(wqkB, [[256, P], [P * 256, CT],
                                               [1, 256]]))
            # gpsimd: x2
            nc.gpsimd.dma_start(out=xt[2], in_=xT[2 * P:3 * P, :])

            # ---------------- constants ----------------------------------
            eps_t = consts.tile([P, 1], FP32, name="eps")
            nc.vector.memset(eps_t, GN_EPS)
            sq_t = consts.tile([P, 1], FP32, name="sq_t")
            # dummy sqrt: pull the sqrt-set ACT table load to ~7us, off
            # the per-tile rstd critical path
            nc.scalar.activation(out=sq_t, in_=eps_t, func=AF.Sqrt,
                                 scale=1.0)
            warm_src = consts.tile([P, 512], BF16, name="warm_src")
            nc.vector.memset(warm_src, 0.125)
            G, GT = _build_group_mats(nc, consts)
            ones64 = consts.tile([HD + 1, HD], BF16, name="ones64")
            nc.vector.memset(ones64, 1.0)

            # PE warmup: one 7-matmul accumulation chain on an "sc" buffer,
            # gated only on the warm_src memset, so the HAM clock gate
            # releases (~3.4us of PE busy) before the first real matmuls.
            warm = ps_sc("warm")
            for i in range(7):
                _mm(nc, warm[:, 0:512], warm_src[:, 0:P], warm_src,
                    i == 0, i == 6)

            # ---------------- per-tile GroupNorm -------------------------
            # Groups (16ch) never cross 128-channel tiles: each tile's
            # stats -> group combine -> alpha/beta pipeline is independent
            # and chases its own DMA.  Stats: DVE bn_stats (tiles 0,2,3),
            # ACT Copy/Square+accum (tile 1).  mv = (mean, var|E2, mean^2);
            # group var = avg(col1+col2) - gmean^2 either way.
            act_junk = xts.tile([P, N], FP32, name="act_junk")
            alpha = [None] * CT
            beta = [None] * CT
            rstds = [None] * CT

            def emit_gn_tile(k):
                mv = small.tile([P, 3], FP32, name=f"mv{k}")
                if k == 1:
                    nc.scalar.activation(
                        out=act_junk, in_=xt[k], func=AF.Copy,
                        scale=1.0 / N, accum_out=mv[:, 0:1])
                    nc.scalar.activation(
                        out=act_junk, in_=xt[k], func=AF.Square,
                        scale=1.0 / 32, accum_out=mv[:, 1:2])
                    nc.vector.memset(mv[:, 2:3], 0.0)
                else:
                    st = small.tile([P, 2, 6], FP32, name=f"bnst{k}")
                    nc.vector.bn_stats(out=st[:, 0, :], in_=xt[k][:, 0:512])
                    nc.vector.bn_stats(out=st[:, 1, :],
                                       in_=xt[k][:, 512:1024])
                    nc.vector.bn_aggr(out=mv[:, 0:2], in_=st)
                    nc.vector.tensor_mul(mv[:, 2:3], mv[:, 0:1], mv[:, 0:1])
                gps = ps_o(f"gps{k}", [8, 3])
                _mm(nc, gps, G, mv, True, True)
                gsb_k = small.tile([8, 3], FP32, name=f"gsbk{k}")
                nc.vector.tensor_copy(gsb_k, gps)
                mvx_ps = ps_o(f"mvx{k}", [P, 3])
                _mm(nc, mvx_ps, GT, gsb_k, True, True)
                mvx = small.tile([P, 3], FP32, name=f"mvxs{k}")
                nc.vector.tensor_copy(mvx, mvx_ps)
                gv = small.tile([P, 1], FP32, name=f"gv{k}")
                gm2 = small.tile([P, 1], FP32, name=f"gm2{k}")
                nc.vector.tensor_add(gv, mvx[:, 1:2], mvx[:, 2:3])
                nc.vector.tensor_mul(gm2, mvx[:, 0:1], mvx[:, 0:1])
                nc.vector.tensor_sub(gv, gv, gm2)      # group var
                # rstd = 1/sqrt(var + eps): ACT Sqrt (sqrt set, loaded
                # once early) + DVE reciprocal.  Ln+Exp would thrash the
                # table RAM: the set chooser picks natural_log for Ln and
                # exp_and_others for Exp (1.3us reload per switch).
                sq = small.tile([P, 1], FP32, name=f"sq{k}")
                nc.scalar.activation(out=sq, in_=gv, func=AF.Sqrt,
                                     bias=eps_t, scale=1.0)
                rstd = small.tile([P, 1], FP32, name=f"rstd{k}")
                nc.vector.reciprocal(rstd, sq)
                rstds[k] = rstd
                a_k = small.tile([P, 1], FP32, name=f"a{k}")
                b_k = small.tile([P, 1], FP32, name=f"b{k}")
                t_k = small.tile([P, 1], FP32, name=f"t{k}")
                nc.vector.tensor_mul(a_k, rstd, gsb_sb[:, 0, k:k + 1])
                nc.vector.tensor_mul(t_k, mvx[:, 0:1], a_k)
                nc.vector.tensor_sub(b_k, gsb_sb[:, 1, k:k + 1], t_k)
                alpha[k], beta[k] = a_k, b_k

            for k in range(CT):
                emit_gn_tile(k)

            # dummy exp, data-gated on the LAST two tiles' rstds: forces
            # the exp-set table load right after the final Sqrt (instead
            # of just before the first stream exp), without re-thrashing
            # the sqrt set.
            rgate = small.tile([P, 1], FP32, name="rgate")
            nc.vector.tensor_add(rgate, rstds[2], rstds[3])
            nc.scalar.activation(out=sq_t, in_=rgate, func=AF.Exp,
                                 scale=1.0)

            # ---------------- xn + qk pair-0 projection ------------------
            # xn on alternating engines so the per-tile chains overlap.
            xn = []
            qk_m0 = ps_sc("qk_m0")     # q01 for all tokens
            qk_m1 = ps_sc("qk_m1")     # k01 for all tokens
            for k in range(CT):
                xnk = xts.tile([P, N], BF16, name=f"xn{k}")
                eng = nc.gpsimd if k in (0, 2) else nc.vector
                eng.tensor_scalar(
                    out=xnk, in0=xt[k],
                    scalar1=alpha[k], scalar2=beta[k],
                    op0=mybir.AluOpType.mult, op1=mybir.AluOpType.add)
                xn.append(xnk)
                for ih in range(2):
                    _mm(nc, qk_m1[:, ih * 512:(ih + 1) * 512],
                        wqkA_sb[:, k, P:2 * P],
                        xnk[:, ih * 512:(ih + 1) * 512],
                        k == 0, k == CT - 1)
                for ih in range(2):
                    _mm(nc, qk_m0[:, ih * 512:(ih + 1) * 512],
                        wqkA_sb[:, k, 0:P],
                        xnk[:, ih * 512:(ih + 1) * 512],
                        k == 0, k == CT - 1)

            qq = [qkpool.tile([P, N], BF16, name=f"qq{p}") for p in range(2)]
            kk = [qkpool.tile([P, N], BF16, name=f"kk{p}") for p in range(2)]

            # kk head-block first (ACT, tiny) so the first scores gate
            # shallow; qq full on DVE; kk rest on ACT.
            nc.scalar.copy(kk[0][:, 0:P], qk_m1[:, 0:P])
            nc.vector.tensor_copy(qq[0], qk_m0)
            nc.scalar.copy(kk[0][:, P:N], qk_m1[:, P:N])

            # ---------------- V projection (prologue, "o" banks) ----------
            v1 = []
            for t in range(TT):
                pv = ps_o(f"pv{t}", [P, NHC * HD])
                for k in range(CT):
                    _mm(nc, pv, xn[k][:, t * P:(t + 1) * P], wv_sb[:, k, :],
                        k == 0, k == CT - 1)
                vt = vpool.tile([P, NHC, HD + 1], BF16, name=f"v1_{t}")
                nc.vector.tensor_copy(
                    vt[:, :, 0:HD], pv.rearrange("p (h d) -> p h d", d=HD))
                nc.vector.memset(vt[:, :, HD:HD + 1], 1.0)
                v1.append(vt)

            # ------- pair-1 qk projection chunks ("sc" steals) ------------
            # Emitted in PAIRS right after a score slot so the sc-rotation
            # parity is preserved (scores keep prefilling one exp ahead).
            def emit_qk23_chunk(c):
                m, ih = c // 2, c % 2
                dst = qq[1] if m == 0 else kk[1]
                qp = ps_sc(f"qk23_{c}", [P, 512])
                for k in range(CT):
                    _mm(nc, qp, wqkB_sb[:, k, m * P:(m + 1) * P],
                        xn[k][:, ih * 512:(ih + 1) * 512],
                        k == 0, k == CT - 1)
                # evac on DVE only: an ACT copy here would preempt the
                # exp stream (scheduler favors earlier-emitted ready ops)
                nc.vector.tensor_copy(dst[:, ih * 512:(ih + 1) * 512], qp)

            # ---- drains: zero-DMA normalize ------------------------------
            # Denominator row (psum row 64) broadcast across 64 partitions
            # with a K=1 PE matmul into a [64,1024] psum (both ih halves),
            # reciprocal_approx_fast, one [64,1024] TT mul into resT.
            resT = [respool.tile([P, N], BF16, name=f"res{p}")
                    for p in range(2)]
            o_sb_hold = [None] * NHC

            def drain_copies(h, tail=False):
                o0, o1 = o_hold[h]
                o_sb = osbpool.tile([HD + 1, N], BF16, name=f"o_sb{h}")
                nc.vector.tensor_copy(o_sb[:, 0:512], o0)
                if tail:
                    nc.scalar.copy(o_sb[:, 512:1024], o1)
                else:
                    nc.vector.tensor_copy(o_sb[:, 512:1024], o1)
                o_sb_hold[h] = o_sb

            def drain_finish(h, dbc):
                # dbc: [64, 1024] psum tile (caller allocates on a tag slot
                # that preserves rotation parity at that point).
                p, q = divmod(h, 2)
                o_sb = o_sb_hold[h]
                for ih in range(2):
                    _mm(nc, dbc[:, ih * 512:(ih + 1) * 512],
                        ones64[HD:HD + 1, :],
                        o_sb[HD:HD + 1, ih * 512:(ih + 1) * 512],
                        True, True, tile_position=(HD, 0))
                rdb = small.tile([HD, N], FP32, name=f"rdb{h}")
                nc.vector.reciprocal_approx_fast(out=rdb, in_=dbc)
                nc.vector.tensor_mul(
                    resT[p][q * HD:(q + 1) * HD, :], o_sb[0:HD, :], rdb)

            # -------- attention: pairs serial, heads interleaved ----------
            o_hold = [None] * NHC
            av_q = []
            av_cnt = [0] * NHC

            def emit_av(h, jt, e_t):
                first = av_cnt[h] == 0
                av_cnt[h] += 1
                last = av_cnt[h] == TT
                for ih in range(2):
                    _mm(nc, o_hold[h][ih], v1[jt][:, h, :],
                        e_t[:, ih * 512:(ih + 1) * 512], first, last)

            for pr in range(2):
                o_hold[2 * pr] = [ps_o(f"o{2 * pr}_{ih}", [HD + 1, 512])
                                  for ih in range(2)]
                o_hold[2 * pr + 1] = [ps_o(f"o{2 * pr + 1}_{ih}",
                                           [HD + 1, 512]) for ih in range(2)]
                for jt in range(TT):
                    for q in range(2):
                        h = 2 * pr + q
                        row = q * HD
                        slot = 2 * jt + q
                        sc = ps_sc(f"sc{h}_{jt}")
                        for ih in range(2):
                            _mm(nc, sc[:, ih * 512:(ih + 1) * 512],
                                kk[pr][row:row + HD, jt * P:(jt + 1) * P],
                                qq[pr][row:row + HD,
                                       ih * 512:(ih + 1) * 512],
                                True, True, tile_position=(row, 0))
                        e_t = epool.tile([P, N], BF16, name="e")
                        nc.scalar.activation(out=e_t, in_=sc, func=AF.Exp,
                                             scale=SCALE)
                        av_q.append((h, jt, e_t))
                        if len(av_q) > 2:
                            emit_av(*av_q.pop(0))
                        if pr == 0 and slot in (3, 7):
                            # paired sc-steals keep rotation parity
                            c0 = 0 if slot == 3 else 2
                            emit_qk23_chunk(c0)
                            emit_qk23_chunk(c0 + 1)
                        if pr == 1 and slot == 4:
                            # pair-0 drains: two back-to-back dbc steals
                            # keep the sc rotation parity; by slot 4 the
                            # o_sb evacuations have certainly landed.
                            dbc0 = ps_sc("dbc_h0", [HD, N])
                            dbc1 = ps_sc("dbc_h1", [HD, N])
                            drain_finish(0, dbc0)
                            drain_finish(1, dbc1)
                if pr == 0:
                    # pair-0 A@V tail + evacuations while pair-1 starts
                    while av_q:
                        emit_av(*av_q.pop(0))
                    drain_copies(0)
                    drain_copies(1)

            # ---------------- tail: drain pair 1 + output projection ------
            while av_q:
                emit_av(*av_q.pop(0))
            drain_copies(2)
            drain_copies(3, tail=True)
            dbc2 = ps_sc("dbc_h2", [HD, N])
            dbc3 = ps_sc("dbc_h3", [HD, N])
            drain_finish(2, dbc2)
            drain_finish(3, dbc3)

            # out-proj: yps0/1 on freed sc slots, yps2-5 on freed o slots
            # (resT0 matmuls can pre-run there), yps6/7 reuse after evac.
            yps = []
            for it in range(TT):
                if it in (0, 1):
                    t = ps_sc(f"yps{it}", [P, 512])
                else:
                    t = ps_o(f"yps{it}", [P, 512])
                yps.append(t)
                _mm(nc, t, resT[0][:, it * P:(it + 1) * P], wo_sb[:, 0, :],
                    True, False)
            for it in range(TT):
                _mm(nc, yps[it], resT[1][:, it * P:(it + 1) * P],
                    wo_sb[:, 1, :], False, True)
                ysb = ypool.tile([P, 512], BF16, name="ysb", bufs=4)
                if it % 2 == 0:
                    nc.vector.tensor_copy(ysb, yps[it])
                else:
                    nc.scalar.copy(ysb, yps[it])
                eng = (nc.sync, nc.scalar, nc.gpsimd)[it % 3]
                eng.dma_start(out=y[it * P:(it + 1) * P, :], in_=ysb)
    if compile:
        nc.compile()
        nc.finalize()
    return nc


_CACHE = {}


def _get_program():
    if "nc" not in _CACHE:
        _CACHE["nc"] = build_program()
    return _CACHE["nc"]


def make_in_maps(x, gn_scale, gn_bias, w_qkv, w_out):
    x = np.ascontiguousarray(x, dtype=np.float32)
    w_qkv = np.asarray(w_qkv, dtype=np.float32)
    w_out = np.asarray(w_out, dtype=np.float32)
    gn_scale = np.asarray(gn_scale, dtype=np.float32)
    gn_bias = np.asarray(gn_bias, dtype=np.float32)
    gsb2 = np.ascontiguousarray(np.stack([gn_scale, gn_bias]))
    # per-head column blocks of w_qkv: head h -> [q | k | v] at 3*HD*h
    qcols = [w_qkv[:, 3 * HD * h:3 * HD * h + HD] for h in range(N_HEADS)]
    kcols = [w_qkv[:, 3 * HD * h + HD:3 * HD * h + 2 * HD]
             for h in range(N_HEADS)]
    vcols = [w_qkv[:, 3 * HD * h + 2 * HD:3 * HD * h + 3 * HD]
             for h in range(N_HEADS)]
    in_maps = []
    for cid in range(NCORES):
        b, hg = divmod(cid, 2)
        hs = [4 * hg + l for l in range(NHC)]
        xb = x[b].reshape(N, C)
        wA = np.concatenate(
            [qcols[hs[0]], qcols[hs[1]], kcols[hs[0]], kcols[hs[1]]], axis=1)
        wB = np.concatenate(
            [qcols[hs[2]], qcols[hs[3]], kcols[hs[2]], kcols[hs[3]]], axis=1)
        wvc = np.concatenate([vcols[h] for h in hs], axis=1)
        woc = np.concatenate([w_out[HD * h:HD * (h + 1), :] for h in hs],
                             axis=0)
        in_maps.append({
            "xT": np.ascontiguousarray(xb.T.astype(ml_dtypes.bfloat16)),
            "gsb2": gsb2,
            "wqkA": np.ascontiguousarray(wA.astype(ml_dtypes.bfloat16)),
            "wqkB": np.ascontiguousarray(wB.astype(ml_dtypes.bfloat16)),
            "wv": np.ascontiguousarray(wvc.astype(ml_dtypes.bfloat16)),
            "wo": np.ascontiguousarray(woc.astype(ml_dtypes.bfloat16)),
        })
    return in_maps


def kernel(x, gn_scale, gn_bias, w_qkv, w_out, b_out, _trace=False,
           _trace_kwargs=None):
    x = np.asarray(x, dtype=np.float32)
    b_out = np.asarray(b_out, dtype=np.float32)
    nc = _get_program()
    in_maps = make_in_maps(x, gn_scale, gn_bias, w_qkv, w_out)
    kw = {}
    if _trace:
        kw = dict(trace=True, **(_trace_kwargs or {}))
    res = run_bass_kernel_spmd(nc, in_maps, list(range(NCORES)), **kw)
    _CACHE["last_results"] = res
    out = np.empty((B, N, C), np.float32)
    for b in range(B):
        y0 = res.results[2 * b]["y"].astype(np.float32)
        y1 = res.results[2 * b + 1]["y"].astype(np.float32)
        out[b] = y0 + y1 + x[b].reshape(N, C) + b_out
    return out.reshape(B, HH, WW, C)


# revision 23
# speedup vs baseline: 1.0179x; 1.0179x over previous
"""Trainium2 Bass kernel for nn_AttentionBlock (GroupNorm + 8-head attention).

Sharding: 8 cores = 4 batches x 2 head-groups (4 heads per core).
Each core computes GroupNorm (duplicated within a batch pair), the QKV
projection for its heads, attention, and a partial output projection.
The host sums the two partials per batch and adds bias + residual.

v6 design notes (vs v5: interleaved head pairs, per-tile GN, no swaps):
  - ACT floor per core: 32 exp ACTIVATEs of [128, 1024] ~= 35.6 us.
    Everything else is scheduled to hide under the exp stream.
  - GroupNorm groups (16 ch) never cross 128-channel tiles, so stats /
    alpha / beta / xn are per-tile pipelines chasing the x DMA (3 queues:
    sync, scalar, gpsimd).  Stats: DVE bn_stats for tiles 0,2,3; ACT
    Copy/Square+accum for tile 1.  rstd = exp(-0.5*ln(var+eps)) on ACT so
    the whole kernel uses ONE table set (natural_log_exp_and_others).
  - Heads of a pair are interleaved in the stream: head 2p at PE rows
    0:64, head 2p+1 at rows 64:128 -> adjacent score matmuls overlap in
    distinct row groups with NO partition-swapped copies (v5's 1MB of
    SBUF->SBUF swap DMA is gone).
  - PSUM: tag "sc" 2x[128,1024] (4 banks) carries qk-pair0 psums, score
    tiles, qk23 chunks (paired insertions to keep rotation parity), dbc
    drain broadcasts, and tail yps tiles.  Tag "o" 4x one-bank carries
    GN gps/mvx, v-proj pv, and the 4 A@V accumulators of the live pair.
  - Drains are v5's zero-DMA normalize (ones-row K=1 PE broadcast of the
    denominator row + reciprocal_approx_fast + TT mul), one [64,1024]
    recip/mul per head.
  - Tail: h2 staggered one slot before h3; out-proj per token tile with
    resT0 matmuls pre-run where banks free early; y DMA over 3 queues.
"""

import ml_dtypes
import numpy as np

import concourse.bass as bass
import concourse.bacc as bacc
import concourse.tile as tile
from concourse import mybir
from concourse.bass_utils import run_bass_kernel_spmd

FP32 = mybir.dt.float32
BF16 = mybir.dt.bfloat16

B, HH, WW, C = 4, 32, 32, 512
N = HH * WW              # 1024 tokens
N_HEADS = 8
HD = C // N_HEADS        # 64
N_GROUPS = 32
GS = C // N_GROUPS       # 16 channels per group
GN_EPS = 1e-6
SCALE = C ** -0.5
NHC = 4                  # heads per core
P = 128
CT = C // P              # 4 channel tiles
TT = N // P              # 8 token tiles
NCORES = 8
AF = mybir.ActivationFunctionType


def _mm(nc, out, lhsT, rhs, start, stop, tile_position=None):
    nc.tensor.matmul(out, lhsT, rhs, start=start, stop=stop,
                     tile_position=tile_position)


def _build_group_mats(nc, consts):
    """G [128, 8] with G[c,g] = (c//16 == g)/16, and GT [8, 128] = 1s mask."""
    G = consts.tile([P, 8], FP32, name="G")
    nc.gpsimd.memset(G, 1.0 / GS)
    nc.gpsimd.affine_select(out=G, in_=G, compare_op=mybir.AluOpType.is_ge,
                            fill=0.0, base=0, pattern=[[-GS, 8]],
                            channel_multiplier=1)
    nc.gpsimd.affine_select(out=G, in_=G, compare_op=mybir.AluOpType.is_ge,
                            fill=0.0, base=GS - 1, pattern=[[GS, 8]],
                            channel_multiplier=-1)
    GT = consts.tile([8, P], FP32, name="GT")
    nc.gpsimd.memset(GT, 1.0)
    nc.gpsimd.affine_select(out=GT, in_=GT, compare_op=mybir.AluOpType.is_ge,
                            fill=0.0, base=0, pattern=[[1, P]],
                            channel_multiplier=-GS)
    nc.gpsimd.affine_select(out=GT, in_=GT, compare_op=mybir.AluOpType.is_ge,
                            fill=0.0, base=GS - 1, pattern=[[-1, P]],
                            channel_multiplier=GS)
    return G, GT


def build_program(compile=True):
    nc = bacc.Bacc()
    xT = nc.dram_tensor("xT", [C, N], BF16, kind="ExternalInput").ap()
    gsb2 = nc.dram_tensor("gsb2", [2, C], FP32, kind="ExternalInput").ap()
    wqkA = nc.dram_tensor("wqkA", [C, 256], BF16, kind="ExternalInput").ap()
    wqkB = nc.dram_tensor("wqkB", [C, 256], BF16, kind="ExternalInput").ap()
    wv = nc.dram_tensor("wv", [C, NHC * HD], BF16, kind="ExternalInput").ap()
    wo = nc.dram_tensor("wo", [NHC * HD, C], BF16, kind="ExternalInput").ap()
    y = nc.dram_tensor("y", [N, C], BF16, kind="ExternalOutput").ap()

    with tile.TileContext(nc) as tc:
        with (
            tc.tile_pool(name="consts", bufs=1) as consts,
            tc.tile_pool(name="xts", bufs=1) as xts,
            tc.tile_pool(name="wpool", bufs=1) as wpool,
            tc.tile_pool(name="qk", bufs=1) as qkpool,
            tc.tile_pool(name="vp", bufs=1) as vpool,
            tc.tile_pool(name="ep", bufs=4) as epool,
            tc.tile_pool(name="osb", bufs=2) as osbpool,
            tc.tile_pool(name="small", bufs=1) as small,
            tc.tile_pool(name="res", bufs=1) as respool,
            tc.tile_pool(name="yp", bufs=1) as ypool,
            tc.tile_pool(name="ps", bufs=1, space="PSUM") as ps,
        ):
            # PSUM tags: "sc" 2x[128,1024] = 4 banks, "o" 4x[128,512] = 4.
            def ps_sc(name, shape=None):
                return ps.tile(shape or [P, N], FP32, name=name, tag="sc",
                               bufs=2)

            def ps_o(name, shape=None):
                return ps.tile(shape or [P, 512], FP32, name=name, tag="o",
                               bufs=4)

            # ---------------- input DMAs, 3 queues ------------------------
            xt = [xts.tile([P, N], BF16, name=f"xt{k}") for k in range(CT)]
            gsb_sb = consts.tile([P, 2, CT], FP32, name="gsb_sb")
            wqkA_sb = wpool.tile([P, CT, 256], BF16, name="wqkA_sb")
            wqkB_sb = wpool.tile([P, CT, 256], BF16, name="wqkB_sb")
            wv_sb = wpool.tile([P, CT, NHC * HD], BF16, name="wv_sb")
            wo_sb = wpool.tile([P, 2, C], BF16, name="wo_sb")

            def _ap(t, ap):
                return bass.AP(tensor=t.tensor, offset=t.offset, ap=ap)

            # HWDGE queues carry all of x (SWDGE starts ~2us late);
            # gpsimd carries the late-needed weights.
            # sync: x0, x3h0, x2h0, gsb2, wqkA
            nc.sync.dma_start(out=xt[0], in_=xT[0:P, :])
            nc.sync.dma_start(out=xt[3][:, 0:512], in_=xT[3 * P:4 * P, 0:512])
            nc.sync.dma_start(out=xt[2][:, 0:512], in_=xT[2 * P:3 * P, 0:512])
            nc.sync.dma_start(out=gsb_sb,
                              in_=_ap(gsb2, [[1, P], [C, 2], [P, CT]]))
            nc.sync.dma_start(out=wqkA_sb,
                              in_=_ap(wqkA, [[256, P], [P * 256, CT],
                                             [1, 256]]))
            # scalar: x1, x3h1, x2h1, wv
            nc.scalar.dma_start(out=xt[1], in_=xT[P:2 * P, :])
            nc.scalar.dma_start(out=xt[3][:, 512:1024],
                                in_=xT[3 * P:4 * P, 512:1024])
            nc.scalar.dma_start(out=xt[2][:, 512:1024],
                                in_=xT[2 * P:3 * P, 512:1024])
            nc.scalar.dma_start(out=wv_sb,
                                in_=_ap(wv, [[256, P], [P * 256, CT],
                                             [1, 256]]))
            # gpsimd: wqkB (needed ~4us into the stream), wo (tail)
            nc.gpsimd.dma_start(out=wqkB_sb,
                                in_=_ap(wqkB, [[256, P], [P * 256, CT],
                                               [1, 256]]))
            nc.gpsimd.dma_start(out=wo_sb,
                                in_=_ap(wo, [[C, P], [P * C, 2], [1, C]]))

            # ---------------- constants ----------------------------------
            eps_t = consts.tile([P, 1], FP32, name="eps")
            nc.vector.memset(eps_t, GN_EPS)
            sq_t = consts.tile([P, 1], FP32, name="sq_t")
            # dummy sqrt: pull the sqrt-set ACT table load to ~7us, off
            # the per-tile rstd critical path
            nc.scalar.activation(out=sq_t, in_=eps_t, func=AF.Sqrt,
                                 scale=1.0)
            warm_src = consts.tile([P, 512], BF16, name="warm_src")
            nc.vector.memset(warm_src, 0.125)
            G, GT = _build_group_mats(nc, consts)
            ones64 = consts.tile([HD + 1, HD], BF16, name="ones64")
            nc.vector.memset(ones64, 1.0)

            # PE warmup: one 7-matmul accumulation chain on an "sc" buffer,
            # gated only on the warm_src memset, so the HAM clock gate
            # releases (~3.4us of PE busy) before the first real matmuls.
            warm = ps_sc("warm")
            for i in range(7):
                _mm(nc, warm[:, 0:512], warm_src[:, 0:P], warm_src,
                    i == 0, i == 6)

            # ---------------- per-tile GroupNorm -------------------------
            # Groups (16ch) never cross 128-channel tiles: each tile's
            # stats -> group combine -> alpha/beta pipeline is independent
            # and chases its own DMA.  Stats: DVE bn_stats (tiles 0,2,3),
            # ACT Copy/Square+accum (tile 1).  mv = (mean, var|E2, mean^2);
            # group var = avg(col1+col2) - gmean^2 either way.
            act_junk = xts.tile([P, N], FP32, name="act_junk")
            alpha = [None] * CT
            beta = [None] * CT
            rstds = [None] * CT

            def emit_gn_tile(k):
                mv = small.tile([P, 3], FP32, name=f"mv{k}")
                if k == 1:
                    nc.scalar.activation(
                        out=act_junk, in_=xt[k], func=AF.Copy,
                        scale=1.0 / N, accum_out=mv[:, 0:1])
                    nc.scalar.activation(
                        out=act_junk, in_=xt[k], func=AF.Square,
                        scale=1.0 / 32, accum_out=mv[:, 1:2])
                    nc.vector.memset(mv[:, 2:3], 0.0)
                else:
                    st = small.tile([P, 2, 6], FP32, name=f"bnst{k}")
                    nc.vector.bn_stats(out=st[:, 0, :], in_=xt[k][:, 0:512])
                    nc.vector.bn_stats(out=st[:, 1, :],
                                       in_=xt[k][:, 512:1024])
                    nc.vector.bn_aggr(out=mv[:, 0:2], in_=st)
                    nc.vector.tensor_mul(mv[:, 2:3], mv[:, 0:1], mv[:, 0:1])
                gps = ps_o(f"gps{k}", [8, 3])
                _mm(nc, gps, G, mv, True, True)
                gsb_k = small.tile([8, 3], FP32, name=f"gsbk{k}")
                nc.vector.tensor_copy(gsb_k, gps)
                mvx_ps = ps_o(f"mvx{k}", [P, 3])
                _mm(nc, mvx_ps, GT, gsb_k, True, True)
                mvx = small.tile([P, 3], FP32, name=f"mvxs{k}")
                nc.vector.tensor_copy(mvx, mvx_ps)
                gv = small.tile([P, 1], FP32, name=f"gv{k}")
                gm2 = small.tile([P, 1], FP32, name=f"gm2{k}")
                nc.vector.tensor_add(gv, mvx[:, 1:2], mvx[:, 2:3])
                nc.vector.tensor_mul(gm2, mvx[:, 0:1], mvx[:, 0:1])
                nc.vector.tensor_sub(gv, gv, gm2)      # group var
                # rstd = 1/sqrt(var + eps): ACT Sqrt (sqrt set, loaded
                # once early) + DVE reciprocal.  Ln+Exp would thrash the
                # table RAM: the set chooser picks natural_log for Ln and
                # exp_and_others for Exp (1.3us reload per switch).
                sq = small.tile([P, 1], FP32, name=f"sq{k}")
                nc.scalar.activation(out=sq, in_=gv, func=AF.Sqrt,
                                     bias=eps_t, scale=1.0)
                rstd = small.tile([P, 1], FP32, name=f"rstd{k}")
                nc.vector.reciprocal(rstd, sq)
                rstds[k] = rstd
                a_k = small.tile([P, 1], FP32, name=f"a{k}")
                b_k = small.tile([P, 1], FP32, name=f"b{k}")
                t_k = small.tile([P, 1], FP32, name=f"t{k}")
                nc.vector.tensor_mul(a_k, rstd, gsb_sb[:, 0, k:k + 1])
                nc.vector.tensor_mul(t_k, mvx[:, 0:1], a_k)
                nc.vector.tensor_sub(b_k, gsb_sb[:, 1, k:k + 1], t_k)
                alpha[k], beta[k] = a_k, b_k

            # tile 2 last: its halves land last in the new queue layout
            for k in (0, 1, 3, 2):
                emit_gn_tile(k)

            # dummy exp, data-gated on the LAST two tiles' rstds: forces
            # the exp-set table load right after the final Sqrt (instead
            # of just before the first stream exp), without re-thrashing
            # the sqrt set.
            rgate = small.tile([P, 1], FP32, name="rgate")
            nc.vector.tensor_add(rgate, rstds[2], rstds[3])
            nc.scalar.activation(out=sq_t, in_=rgate, func=AF.Exp,
                                 scale=1.0)

            # ---------------- xn + qk pair-0 projection ------------------
            # xn on alternating engines so the per-tile chains overlap.
            xn = [None] * CT
            qk_m0 = ps_sc("qk_m0")     # q01 for all tokens
            qk_m1 = ps_sc("qk_m1")     # k01 for all tokens
            for i, k in enumerate((0, 1, 3, 2)):
                xnk = xts.tile([P, N], BF16, name=f"xn{k}")
                eng = nc.gpsimd if k in (0, 3) else nc.vector
                eng.tensor_scalar(
                    out=xnk, in0=xt[k],
                    scalar1=alpha[k], scalar2=beta[k],
                    op0=mybir.AluOpType.mult, op1=mybir.AluOpType.add)
                xn[k] = xnk
                for ih in range(2):
                    _mm(nc, qk_m1[:, ih * 512:(ih + 1) * 512],
                        wqkA_sb[:, k, P:2 * P],
                        xnk[:, ih * 512:(ih + 1) * 512],
                        i == 0, i == CT - 1)
                for ih in range(2):
                    _mm(nc, qk_m0[:, ih * 512:(ih + 1) * 512],
                        wqkA_sb[:, k, 0:P],
                        xnk[:, ih * 512:(ih + 1) * 512],
                        i == 0, i == CT - 1)

            qq = [qkpool.tile([P, N], BF16, name=f"qq{p}") for p in range(2)]
            kk = [qkpool.tile([P, N], BF16, name=f"kk{p}") for p in range(2)]

            # kk head-block first (ACT, tiny) so the first scores gate
            # shallow; qq full on DVE; kk rest on ACT.
            nc.scalar.copy(kk[0][:, 0:P], qk_m1[:, 0:P])
            nc.vector.tensor_copy(qq[0], qk_m0)
            nc.scalar.copy(kk[0][:, P:N], qk_m1[:, P:N])

            # ---------------- V projection (prologue, "o" banks) ----------
            v1 = []
            for t in range(TT):
                pv = ps_o(f"pv{t}", [P, NHC * HD])
                for k in range(CT):
                    _mm(nc, pv, xn[k][:, t * P:(t + 1) * P], wv_sb[:, k, :],
                        k == 0, k == CT - 1)
                vt = vpool.tile([P, NHC, HD + 1], BF16, name=f"v1_{t}")
                nc.vector.tensor_copy(
                    vt[:, :, 0:HD], pv.rearrange("p (h d) -> p h d", d=HD))
                nc.vector.memset(vt[:, :, HD:HD + 1], 1.0)
                v1.append(vt)

            # ------- pair-1 qk projection chunks ("sc" steals) ------------
            # Emitted in PAIRS right after a score slot so the sc-rotation
            # parity is preserved (scores keep prefilling one exp ahead).
            def emit_qk23_chunk(c, qp=None):
                m, ih = c // 2, c % 2
                dst = qq[1] if m == 0 else kk[1]
                if qp is None:
                    qp = ps_sc(f"qk23_{c}", [P, 512])
                for k in range(CT):
                    _mm(nc, qp, wqkB_sb[:, k, m * P:(m + 1) * P],
                        xn[k][:, ih * 512:(ih + 1) * 512],
                        k == 0, k == CT - 1)
                # evac on DVE only: an ACT copy here would preempt the
                # exp stream (scheduler favors earlier-emitted ready ops)
                nc.vector.tensor_copy(dst[:, ih * 512:(ih + 1) * 512], qp)

            # ---- drains: zero-DMA normalize ------------------------------
            # Denominator row (psum row 64) broadcast across 64 partitions
            # with a K=1 PE matmul into a [64,1024] psum (both ih halves),
            # reciprocal_approx_fast, one [64,1024] TT mul into resT.
            resT = [respool.tile([P, N], BF16, name=f"res{p}")
                    for p in range(2)]
            o_sb_hold = [None] * NHC

            def drain_copies(h, tail=False):
                o0, o1 = o_hold[h]
                o_sb = osbpool.tile([HD + 1, N], BF16, name=f"o_sb{h}")
                nc.vector.tensor_copy(o_sb[:, 0:512], o0)
                if tail:
                    nc.scalar.copy(o_sb[:, 512:1024], o1)
                else:
                    nc.vector.tensor_copy(o_sb[:, 512:1024], o1)
                o_sb_hold[h] = o_sb

            def drain_finish(h, dbc):
                # dbc: [64, 1024] psum tile (caller allocates on a tag slot
                # that preserves rotation parity at that point).
                p, q = divmod(h, 2)
                o_sb = o_sb_hold[h]
                for ih in range(2):
                    _mm(nc, dbc[:, ih * 512:(ih + 1) * 512],
                        ones64[HD:HD + 1, :],
                        o_sb[HD:HD + 1, ih * 512:(ih + 1) * 512],
                        True, True, tile_position=(HD, 0))
                rdb = small.tile([HD, N], FP32, name=f"rdb{h}")
                nc.vector.reciprocal_approx_fast(out=rdb, in_=dbc)
                nc.vector.tensor_mul(
                    resT[p][q * HD:(q + 1) * HD, :], o_sb[0:HD, :], rdb)

            # qk23 chunks 0,1 on "o" slots (freed by early pv evacs):
            # their matmuls need only wqkB + xn, so they fill the PE
            # during the qq/kk cast window before the stream.
            emit_qk23_chunk(0, qp=ps_o("qk23_0", [P, 512]))
            emit_qk23_chunk(1, qp=ps_o("qk23_1", [P, 512]))

            # -------- attention: pairs serial, heads interleaved ----------
            o_hold = [None] * NHC
            av_q = []
            av_cnt = [0] * NHC

            def emit_av(h, jt, e_t):
                first = av_cnt[h] == 0
                av_cnt[h] += 1
                last = av_cnt[h] == TT
                for ih in range(2):
                    _mm(nc, o_hold[h][ih], v1[jt][:, h, :],
                        e_t[:, ih * 512:(ih + 1) * 512], first, last)

            for pr in range(2):
                o_hold[2 * pr] = [ps_o(f"o{2 * pr}_{ih}", [HD + 1, 512])
                                  for ih in range(2)]
                o_hold[2 * pr + 1] = [ps_o(f"o{2 * pr + 1}_{ih}",
                                           [HD + 1, 512]) for ih in range(2)]
                if pr == 0:
                    slots = [(q, jt) for jt in range(TT) for q in range(2)]
                else:
                    # h2 finishes 2 slots early so its drain chain
                    # (cast -> dbc -> recip -> mul) overlaps the last exps
                    slots = ([(q, jt) for jt in range(6) for q in range(2)]
                             + [(0, 6), (0, 7), (1, 6), (1, 7)])
                for slot, (q, jt) in enumerate(slots):
                    if True:
                        h = 2 * pr + q
                        row = q * HD
                        sc = ps_sc(f"sc{h}_{jt}")
                        for ih in range(2):
                            _mm(nc, sc[:, ih * 512:(ih + 1) * 512],
                                kk[pr][row:row + HD, jt * P:(jt + 1) * P],
                                qq[pr][row:row + HD,
                                       ih * 512:(ih + 1) * 512],
                                True, True, tile_position=(row, 0))
                        e_t = epool.tile([P, N], BF16, name="e")
                        nc.scalar.activation(out=e_t, in_=sc, func=AF.Exp,
                                             scale=SCALE)
                        av_q.append((h, jt, e_t))
                        if len(av_q) > 2:
                            emit_av(*av_q.pop(0))
                        if pr == 0 and slot == 1:
                            # paired sc-steals keep rotation parity; the
                            # 8 matmuls run during exp 0-2
                            emit_qk23_chunk(2)
                            emit_qk23_chunk(3)
                        if pr == 1 and slot == 4:
                            # pair-0 drains: two back-to-back dbc steals
                            # keep the sc rotation parity; by slot 4 the
                            # o_sb evacuations have certainly landed.
                            dbc0 = ps_sc("dbc_h0", [HD, N])
                            dbc1 = ps_sc("dbc_h1", [HD, N])
                            drain_finish(0, dbc0)
                            drain_finish(1, dbc1)
                        if pr == 1 and slot == 15:
                            # h2's last AV just popped: evacuate its o
                            # psums on DVE (ACT still owns the last exps)
                            drain_copies(2)
                if pr == 0:
                    # pair-0 A@V tail + evacuations while pair-1 starts
                    while av_q:
                        emit_av(*av_q.pop(0))
                    drain_copies(0)
                    drain_copies(1)

            # ---------------- tail: drain pair 1 + output projection ------
            while av_q:
                emit_av(*av_q.pop(0))
            drain_copies(3, tail=True)
            dbc2 = ps_sc("dbc_h2", [HD, N])
            dbc3 = ps_sc("dbc_h3", [HD, N])
            drain_finish(2, dbc2)
            drain_finish(3, dbc3)

            # K-split out-proj: resT0 matmuls (K=128) pre-run on freed
            # banks while the pair-1 drain chain finishes; only the K=64
            # h2/h3 matmuls wait on the normalize muls.
            yps = [None] * TT

            def finish_it(it):
                _mm(nc, yps[it], resT[1][:, it * P:(it + 1) * P],
                    wo_sb[:, 1, :], False, True)
                ysb = ypool.tile([P, 512], BF16, name="ysb", bufs=4)
                if it % 2 == 0:
                    nc.vector.tensor_copy(ysb, yps[it])
                else:
                    nc.scalar.copy(ysb, yps[it])
                eng = (nc.sync, nc.scalar, nc.gpsimd)[it % 3]
                eng.dma_start(out=y[it * P:(it + 1) * P, :], in_=ysb)

            for it in range(6):
                t = (ps_sc(f"yps{it}", [P, 512]) if it in (0, 5)
                     else ps_o(f"yps{it}", [P, 512]))
                yps[it] = t
                _mm(nc, t, resT[0][:, it * P:(it + 1) * P], wo_sb[:, 0, :],
                    True, False)
            for it in range(6):
                finish_it(it)
            for it in (6, 7):
                t = ps_sc(f"yps{it}", [P, 512])
                yps[it] = t
                _mm(nc, t, resT[0][:, it * P:(it + 1) * P], wo_sb[:, 0, :],
                    True, False)
                finish_it(it)
    if compile:
        nc.compile()
        nc.finalize()
    return nc


_CACHE = {}


def _get_program():
    if "nc" not in _CACHE:
        _CACHE["nc"] = build_program()
    return _CACHE["nc"]


def make_in_maps(x, gn_scale, gn_bias, w_qkv, w_out):
    x = np.ascontiguousarray(x, dtype=np.float32)
    w_qkv = np.asarray(w_qkv, dtype=np.float32)
    w_out = np.asarray(w_out, dtype=np.float32)
    gn_scale = np.asarray(gn_scale, dtype=np.float32)
    gn_bias = np.asarray(gn_bias, dtype=np.float32)
    gsb2 = np.ascontiguousarray(np.stack([gn_scale, gn_bias]))
    # per-head column blocks of w_qkv: head h -> [q | k | v] at 3*HD*h
    qcols = [w_qkv[:, 3 * HD * h:3 * HD * h + HD] for h in range(N_HEADS)]
    kcols = [w_qkv[:, 3 * HD * h + HD:3 * HD * h + 2 * HD]
             for h in range(N_HEADS)]
    vcols = [w_qkv[:, 3 * HD * h + 2 * HD:3 * HD * h + 3 * HD]
             for h in range(N_HEADS)]
    in_maps = []
    for cid in range(NCORES):
        b, hg = divmod(cid, 2)
        hs = [4 * hg + l for l in range(NHC)]
        xb = x[b].reshape(N, C)
        wA = np.concatenate(
            [qcols[hs[0]], qcols[hs[1]], kcols[hs[0]], kcols[hs[1]]], axis=1)
        wB = np.concatenate(
            [qcols[hs[2]], qcols[hs[3]], kcols[hs[2]], kcols[hs[3]]], axis=1)
        wvc = np.concatenate([vcols[h] for h in hs], axis=1)
        woc = np.concatenate([w_out[HD * h:HD * (h + 1), :] for h in hs],
                             axis=0)
        in_maps.append({
            "xT": np.ascontiguousarray(xb.T.astype(ml_dtypes.bfloat16)),
            "gsb2": gsb2,
            "wqkA": np.ascontiguousarray(wA.astype(ml_dtypes.bfloat16)),
            "wqkB": np.ascontiguousarray(wB.astype(ml_dtypes.bfloat16)),
            "wv": np.ascontiguousarray(wvc.astype(ml_dtypes.bfloat16)),
            "wo": np.ascontiguousarray(woc.astype(ml_dtypes.bfloat16)),
        })
    return in_maps


def kernel(x, gn_scale, gn_bias, w_qkv, w_out, b_out, _trace=False,
           _trace_kwargs=None):
    x = np.asarray(x, dtype=np.float32)
    b_out = np.asarray(b_out, dtype=np.float32)
    nc = _get_program()
    in_maps = make_in_maps(x, gn_scale, gn_bias, w_qkv, w_out)
    kw = {}
    if _trace:
        kw = dict(trace=True, **(_trace_kwargs or {}))
    res = run_bass_kernel_spmd(nc, in_maps, list(range(NCORES)), **kw)
    _CACHE["last_results"] = res
    out = np.empty((B, N, C), np.float32)
    for b in range(B):
        y0 = res.results[2 * b]["y"].astype(np.float32)
        y1 = res.results[2 * b + 1]["y"].astype(np.float32)
        out[b] = y0 + y1 + x[b].reshape(N, C) + b_out
    return out.reshape(B, HH, WW, C)


# revision 24
# speedup vs baseline: 1.0981x; 1.0789x over previous
"""Trainium2 Bass kernel for nn_AttentionBlock (GroupNorm + 8-head attention).

Sharding: 8 cores = 4 batches x 2 head-groups (4 heads per core).
Each core computes GroupNorm (duplicated within a batch pair), the QKV
projection for its heads, attention, and a partial output projection.
The host sums the two partials per batch and adds bias + residual.

v5 design notes (ACT-engine-bound schedule):
  - Hard floor per core: softmax exp = 32 ACTIVATE ops of [128, 1024]
    at (N+352)/1.2 ns ~= 36.7 us on the Scalar engine; everything else
    is scheduled to hide under it.
  - DMA reality on this part: ~0.65us issue per descriptor and ~1-3us
    completion latency; queues serialize.  All input loads go on the
    sync queue in priority order (x first, in half-tiles so bn_stats
    can chase), weights after.
  - Engines are strict in-order FIFOs: any op that waits on a DMA
    stalls everything behind it on that engine.  Drains are therefore
    pipelined across head phases: stage A (psum evacuation + D-row
    repartition DMA) at phase h+1, stage B (reciprocal on DVE - its
    DMA dependency long satisfied - then DRAM-roundtrip broadcast and
    the normalize multiply ON GPSIMD, whose stalls are harmless) a
    phase later.
  - Score matmuls are K=64: jt-even runs in the head's own PE row
    group, jt-odd concurrently in the opposite group via
    partition-swapped copies of qq/kk (SBUF->SBUF DMA).
  - Head 3 is never normalized: 1/D3 is repartitioned (one DMA) and
    folded into the output combine as a per-partition scalar via
    scalar_tensor_tensor (token index lives on the psum partition).
  - PSUM: tag "sc" 2x[128,1024] (4 banks), tag "o" 3x one-bank
    (v-proj + A@V accumulators), tag "sp" 1x[128,512].
"""

import ml_dtypes
import numpy as np

import concourse.bass as bass
import concourse.bacc as bacc
import concourse.tile as tile
from concourse import mybir
from concourse.bass_utils import run_bass_kernel_spmd

FP32 = mybir.dt.float32
BF16 = mybir.dt.bfloat16

B, HH, WW, C = 4, 32, 32, 512
N = HH * WW              # 1024 tokens
N_HEADS = 8
HD = C // N_HEADS        # 64
N_GROUPS = 32
GS = C // N_GROUPS       # 16 channels per group
GN_EPS = 1e-6
SCALE = C ** -0.5
NHC = 4                  # heads per core
P = 128
CT = C // P              # 4 channel tiles
TT = N // P              # 8 token tiles
NCORES = 8


def _mm(nc, out, lhsT, rhs, start, stop, tile_position=None):
    nc.tensor.matmul(out, lhsT, rhs, start=start, stop=stop,
                     tile_position=tile_position)


def _build_group_mats(nc, consts):
    """G [128, 8] with G[c,g] = (c//16 == g)/16, and GT [8, 128] = 1s mask."""
    G = consts.tile([P, 8], FP32, name="G")
    nc.gpsimd.memset(G, 1.0 / GS)
    nc.gpsimd.affine_select(out=G, in_=G, compare_op=mybir.AluOpType.is_ge,
                            fill=0.0, base=0, pattern=[[-GS, 8]],
                            channel_multiplier=1)
    nc.gpsimd.affine_select(out=G, in_=G, compare_op=mybir.AluOpType.is_ge,
                            fill=0.0, base=GS - 1, pattern=[[GS, 8]],
                            channel_multiplier=-1)
    GT = consts.tile([8, P], FP32, name="GT")
    nc.gpsimd.memset(GT, 1.0)
    nc.gpsimd.affine_select(out=GT, in_=GT, compare_op=mybir.AluOpType.is_ge,
                            fill=0.0, base=0, pattern=[[1, P]],
                            channel_multiplier=-GS)
    nc.gpsimd.affine_select(out=GT, in_=GT, compare_op=mybir.AluOpType.is_ge,
                            fill=0.0, base=GS - 1, pattern=[[-1, P]],
                            channel_multiplier=GS)
    return G, GT


def build_program(compile=True):
    nc = bacc.Bacc()
    xT = nc.dram_tensor("xT", [C, N], BF16, kind="ExternalInput").ap()
    wqk = nc.dram_tensor("wqk", [C, 512], BF16, kind="ExternalInput").ap()
    wv = nc.dram_tensor("wv", [C, NHC * HD], BF16, kind="ExternalInput").ap()
    wo = nc.dram_tensor("wo", [NHC * HD, C], BF16, kind="ExternalInput").ap()
    gsc = nc.dram_tensor("gsc", [C], FP32, kind="ExternalInput").ap()
    gbi = nc.dram_tensor("gbi", [C], FP32, kind="ExternalInput").ap()
    y = nc.dram_tensor("y", [N, C], BF16, kind="ExternalOutput").ap()

    with tile.TileContext(nc) as tc:
        with (
            tc.tile_pool(name="consts", bufs=1) as consts,
            tc.tile_pool(name="xts", bufs=1) as xts,
            tc.tile_pool(name="wpool", bufs=1) as wpool,
            tc.tile_pool(name="qk", bufs=1) as qkpool,
            tc.tile_pool(name="vp", bufs=1) as vpool,
            tc.tile_pool(name="ep", bufs=5) as epool,
            tc.tile_pool(name="osb", bufs=2) as osbpool,
            tc.tile_pool(name="small", bufs=1) as small,
            tc.tile_pool(name="res", bufs=1) as respool,
            tc.tile_pool(name="yp", bufs=1) as ypool,
            tc.tile_pool(name="ps", bufs=1, space="PSUM") as ps,
        ):
            # PSUM tags: sc 2x[128,1024]=4 banks, o 3x 1 bank, sp 1 bank.
            def ps_sc(name):
                return ps.tile([P, N], FP32, name=name, tag="sc", bufs=2)

            def ps_o(name):
                return ps.tile([HD + 1, 512], FP32, name=name, tag="o",
                               bufs=3)

            def ps_v(name):
                return ps.tile([P, NHC * HD], FP32, name=name, tag="o",
                               bufs=3)

            def ps_sp(name):
                return ps.tile([P, 512], FP32, name=name, tag="sp", bufs=1)

            eps_t = consts.tile([P, 1], FP32, name="eps")
            nc.vector.memset(eps_t, GN_EPS)
            sq_t = consts.tile([P, 1], FP32, name="sq_t")
            # dummy sqrt: pull the ACT table load off the GN critical path
            nc.scalar.activation(out=sq_t, in_=eps_t,
                                 func=mybir.ActivationFunctionType.Sqrt,
                                 scale=1.0)

            # -------- input DMAs: ONE queue, priority order --------------
            xt = []
            for k in range(CT):
                t = xts.tile([P, N], BF16, name=f"xt{k}")
                xt.append(t)
            # x tiles 0-1 stream on the sync queue, 2-3 in parallel on the
            # scalar queue (per-queue transfer rate ~107GB/s is the limit,
            # not HBM) so the last tile lands ~1.5us earlier
            for k in range(CT):
                eng = nc.sync if k < 2 else nc.scalar
                for hh in range(2):
                    eng.dma_start(out=xt[k][:, hh * 512:(hh + 1) * 512],
                                  in_=xT[k * P:(k + 1) * P,
                                         hh * 512:(hh + 1) * 512])
            gs4 = consts.tile([P, CT], FP32, name="gs4")
            gb4 = consts.tile([P, CT], FP32, name="gb4")
            nc.sync.dma_start(
                out=gs4, in_=bass.AP(tensor=gsc.tensor, offset=gsc.offset,
                                     ap=[[1, P], [P, CT]]))
            nc.sync.dma_start(
                out=gb4, in_=bass.AP(tensor=gbi.tensor, offset=gbi.offset,
                                     ap=[[1, P], [P, CT]]))
            wqk_sb = []
            for k in range(CT):
                t = wpool.tile([P, 512], BF16, name=f"wqk{k}")
                nc.sync.dma_start(out=t, in_=wqk[k * P:(k + 1) * P, :])
                wqk_sb.append(t)
            wv_sb = []
            for k in range(CT):
                t = wpool.tile([P, NHC * HD], BF16, name=f"wv{k}")
                nc.sync.dma_start(out=t, in_=wv[k * P:(k + 1) * P, :])
                wv_sb.append(t)
            wo_sb = []
            for p in range(2):
                t = wpool.tile([P, 512], BF16, name=f"wo{p}")
                nc.sync.dma_start(out=t, in_=wo[p * P:(p + 1) * P, :])
                wo_sb.append(t)

            G, GT = _build_group_mats(nc, consts)

            # PE warmups: must be a PSUM-accumulation CHAIN (independent
            # start/stop matmuls serialize on buffer-reuse deps at ~640ns
            # and never release the HAM clock gate), on the "o" tag so the
            # shared-buffer WAW never delays the GroupNorm matmuls ("sp").
            def warm_chain(tag, n):
                warm = ps.tile([P, 512], FP32, name=f"warm_{tag}", tag="o",
                               bufs=3)
                for i in range(n):
                    _mm(nc, warm, xt[0][:, 0:P], xt[0][:, 0:512],
                        i == 0, i == n - 1)

            warm_chain("a", 8)

            # ---------------- GroupNorm stats ----------------
            # per-k group combine so only the last tile's chain is on the
            # critical path; warmup matmuls bridge the DVE-heavy window.
            # The group combine only needs (mean, E[x^2]) per channel (law
            # of total variance): tile 3's stats run on the idle ACT engine
            # as Copy/Square with accum_out, feeding (mean, E[x^2], 0)
            # while the DVE handles tiles 0-2 with bn_stats.
            mv = small.tile([P, CT, 3], FP32, name="mv")
            act_junk = xts.tile([P, N], FP32, name="act_junk")
            gps = ps.tile([8, 3 * CT], FP32, name="gps", tag="sp", bufs=1)
            for k in range(CT):
                if k < 3:
                    st = small.tile([P, 2, 6], FP32, name=f"bnst{k}")
                    nc.vector.bn_stats(out=st[:, 0, :], in_=xt[k][:, 0:512])
                    nc.vector.bn_stats(out=st[:, 1, :],
                                       in_=xt[k][:, 512:1024])
                    nc.vector.bn_aggr(out=mv[:, k, 0:2], in_=st)
                    nc.vector.tensor_mul(mv[:, k, 2:3], mv[:, k, 0:1],
                                         mv[:, k, 0:1])
                else:
                    nc.scalar.activation(
                        out=act_junk, in_=xt[k],
                        func=mybir.ActivationFunctionType.Copy,
                        scale=1.0 / N, accum_out=mv[:, k, 0:1])
                    nc.scalar.activation(
                        out=act_junk, in_=xt[k],
                        func=mybir.ActivationFunctionType.Square,
                        scale=1.0 / 32, accum_out=mv[:, k, 1:2])
                    nc.vector.memset(mv[:, k, 2:3], 0.0)
                _mm(nc, gps[:, 3 * k:3 * k + 3], G, mv[:, k, :],
                    k == 0, k == CT - 1)
                if k in (1, 3):
                    warm_chain(f"k{k}", 6)
            gsb = consts.tile([8, 3 * CT], FP32, name="gsb")
            nc.vector.tensor_copy(gsb, gps)
            mvx_ps = ps.tile([P, 3 * CT], FP32, name="mvx_ps", tag="sp",
                             bufs=1)
            _mm(nc, mvx_ps, GT, gsb, True, True)
            mvx = consts.tile([P, CT, 3], FP32, name="mvx")
            nc.vector.tensor_copy(mvx, mvx_ps.rearrange("p (k s) -> p k s",
                                                        s=3))
            # PE warmup part 2: bridge the GN-combine window.  Data-gated
            # on gsb so the scheduler cannot run it before the GT matmul's
            # input is ready (and thus cannot displace the GT matmul).
            gsb_bf = consts.tile([8, 3 * CT], BF16, name="gsb_bf")
            nc.vector.tensor_copy(gsb_bf, gps)
            warmb = ps.tile([P, 512], FP32, name="warmb", tag="o", bufs=3)
            # tiny gate matmul depends on gsb, so the full-width bridge
            # chain behind it (same buffer, in-order PE) cannot be hoisted
            # ahead of the GT matmul by the scheduler
            _mm(nc, warmb[0:3 * CT, 0:64], gsb_bf, xt[0][0:8, 0:64],
                True, True)
            for i in range(14):
                _mm(nc, warmb, xt[0][:, 0:P], xt[0][:, 0:512],
                    i == 0, i == 13)
            t4 = consts.tile([P, CT], FP32, name="t4")
            v4 = consts.tile([P, CT], FP32, name="v4")
            ab = consts.tile([P, CT, 2], FP32, name="ab")
            m4 = mvx[:, :, 0]
            nc.vector.tensor_add(t4, mvx[:, :, 1], mvx[:, :, 2])
            nc.vector.tensor_mul(v4, m4, m4)
            nc.vector.tensor_sub(v4, t4, v4)          # group var per channel
            nc.scalar.activation(out=v4, in_=v4,
                                 func=mybir.ActivationFunctionType.Sqrt,
                                 bias=eps_t, scale=1.0)
            # dummy exp straight after the sqrt (data-gated on its output so
            # the scheduler cannot hoist it): forces the exp table load now,
            # while ACT is idle, instead of right before the first real exp
            nc.scalar.activation(out=sq_t, in_=v4[:, 0:1],
                                 func=mybir.ActivationFunctionType.Exp,
                                 scale=1.0)
            nc.vector.reciprocal(v4, v4)              # rstd per channel
            nc.vector.tensor_mul(ab[:, :, 0], v4, gs4)           # alpha
            nc.vector.tensor_mul(t4, m4, ab[:, :, 0])
            nc.vector.tensor_sub(ab[:, :, 1], gb4, t4)           # beta

            # ---------------- xn + pair-0 qk projection -------------------
            # qk_m1 (kk) goes to two one-bank "o"-tag psums so both "sc"
            # buffers are free the moment qq's casts finish: the first TWO
            # score tiles then gate only on the cheap early casts.
            xn = []
            qk_m0 = ps_sc("qk_m0")
            qk_m1 = [ps.tile([P, 512], FP32, name=f"qk_m1_{ih}", tag="o",
                             bufs=3) for ih in range(2)]
            for k in range(CT):
                xnk = xts.tile([P, N], BF16, name=f"xn{k}")
                eng = nc.gpsimd if k == 3 else nc.vector
                eng.tensor_scalar(
                    out=xnk, in0=xt[k],
                    scalar1=ab[:, k, 0:1], scalar2=ab[:, k, 1:2],
                    op0=mybir.AluOpType.mult, op1=mybir.AluOpType.add)
                xn.append(xnk)
                for ih in range(2):
                    _mm(nc, qk_m1[ih],
                        wqk_sb[k][:, P:2 * P],
                        xnk[:, ih * 512:(ih + 1) * 512],
                        k == 0, k == CT - 1)
                for ih in range(2):
                    _mm(nc, qk_m0[:, ih * 512:(ih + 1) * 512],
                        wqk_sb[k][:, 0:P],
                        xnk[:, ih * 512:(ih + 1) * 512],
                        k == 0, k == CT - 1)

            qq = [qkpool.tile([P, N], BF16, name=f"qq{p}") for p in range(2)]
            kk = [qkpool.tile([P, N], BF16, name=f"kk{p}") for p in range(2)]
            qqs = [qkpool.tile([P, N], BF16, name=f"qqs{p}") for p in range(2)]
            kks = [qkpool.tile([P, N], BF16, name=f"kks{p}") for p in range(2)]
            resT = [respool.tile([P, N], BF16, name=f"res{p}")
                    for p in range(2)]

            def swap_dma(dst, src):
                nc.gpsimd.dma_start(out=dst[0:HD, :], in_=src[HD:P, :])
                nc.gpsimd.dma_start(out=dst[HD:P, :], in_=src[0:HD, :])

            # the first exp needs BOTH qq halves (the sc tile spans both ih
            # blocks) but only kk columns 0:128 (jt 0); order casts so that
            # gate is as shallow as possible, with ACT carrying one half
            nc.vector.tensor_copy(kk[0][:, 0:P], qk_m1[0][:, 0:P])
            nc.scalar.copy(qq[0][:, 0:512], qk_m0[:, 0:512])
            nc.vector.tensor_copy(qq[0][:, 512:1024], qk_m0[:, 512:1024])
            nc.vector.tensor_copy(kk[0][:, P:512], qk_m1[0][:, P:512])
            nc.vector.tensor_copy(kk[0][:, 512:1024], qk_m1[1])
            swap_dma(kks[0], kk[0])
            swap_dma(qqs[0], qq[0])

            # ---------------- V projection ----------------
            v1 = []
            for t in range(TT):
                pv = ps_v(f"pv{t}")
                for k in range(CT):
                    _mm(nc, pv, xn[k][:, t * P:(t + 1) * P], wv_sb[k],
                        k == 0, k == CT - 1)
                vt = vpool.tile([P, NHC, HD + 1], BF16, name=f"v1_{t}")
                nc.vector.tensor_copy(
                    vt[:, :, 0:HD], pv.rearrange("p (h d) -> p h d", d=HD))
                nc.vector.memset(vt[:, :, HD:HD + 1], 1.0)
                v1.append(vt)

            # ------- pair-1 qk projection, in [128,512] chunks on "sp" ----
            # emitted one chunk at a time inside head 0's late jts so the
            # in-order PE FIFO never blocks head 1's first scores
            def emit_qk23_chunk(c):
                m, ih = 2 + c // 2, c % 2
                dst = qq[1] if m == 2 else kk[1]
                qp = ps_sp(f"qk{m}_{ih}")
                for k in range(CT):
                    _mm(nc, qp, wqk_sb[k][:, m * P:(m + 1) * P],
                        xn[k][:, ih * 512:(ih + 1) * 512],
                        k == 0, k == CT - 1)
                nc.vector.tensor_copy(dst[:, ih * 512:(ih + 1) * 512], qp)
                if c == 1:
                    swap_dma(qqs[1], qq[1])
                if c == 3:
                    swap_dma(kks[1], kk[1])

            # ---- drains: zero-DMA normalize ------------------------------
            # The softmax denominator row (psum row 64) is broadcast across
            # 64 partitions with a K=1 PE matmul (ones row as stationary),
            # inverted with reciprocal_approx_fast, and multiplied in.  No
            # DMA anywhere, so no engine ever stalls on a DMA completion.
            ones64 = consts.tile([HD + 1, HD], BF16, name="ones64")
            nc.vector.memset(ones64, 1.0)
            o_sb_hold = [None] * NHC

            def drain_copies(h, tail=False):
                o0, o1 = o_hold[h]
                o_sb = osbpool.tile([HD + 1, N], BF16, name=f"o_sb{h}")
                nc.vector.tensor_copy(o_sb[:, 0:512], o0)
                if tail:
                    nc.scalar.copy(o_sb[:, 512:1024], o1)
                else:
                    nc.vector.tensor_copy(o_sb[:, 512:1024], o1)
                o_sb_hold[h] = o_sb

            def drain_finish(h):
                p, q = divmod(h, 2)
                o_sb = o_sb_hold[h]
                for ih in range(2):
                    dbc = ps.tile([HD, 512], FP32, name=f"dbc{h}_{ih}",
                                  tag="sp", bufs=1)
                    _mm(nc, dbc, ones64[HD:HD + 1, :],
                        o_sb[HD:HD + 1, ih * 512:(ih + 1) * 512],
                        True, True, tile_position=(HD, 0))
                    rdb = small.tile([HD, 512], FP32, name=f"rdb{h}_{ih}")
                    nc.vector.reciprocal_approx_fast(out=rdb, in_=dbc)
                    nc.vector.tensor_mul(
                        resT[p][q * HD:(q + 1) * HD,
                                ih * 512:(ih + 1) * 512],
                        o_sb[0:HD, ih * 512:(ih + 1) * 512], rdb)

            # -------- attention: heads serial, ACT-rate pipeline ----------
            o_hold = [None] * NHC
            for h in range(NHC):
                p, q = divmod(h, 2)
                if h >= 1:
                    drain_copies(h - 1)
                o_hold[h] = [ps_o(f"o{h}_0"), ps_o(f"o{h}_1")]
                # head 0 does even jts first: the odd-jt stream depends on
                # the partition-swapped copies, which arrive a bit later
                jts = [0, 2, 4, 6, 1, 3, 5, 7] if h == 0 else range(TT)
                # A@V for jt is emitted AFTER the next jt's score matmuls so
                # the in-order PE FIFO overlaps scores with the exp latency
                av_q = []
                for n_jt, jt in enumerate(jts):
                    sc = ps_sc(f"sc{h}_{jt}")
                    if jt % 2 == 0:
                        lk, lq, row = kk[p], qq[p], q * HD
                    else:
                        lk, lq, row = kks[p], qqs[p], (1 - q) * HD
                    for ih in range(2):
                        _mm(nc, sc[:, ih * 512:(ih + 1) * 512],
                            lk[row:row + HD, jt * P:(jt + 1) * P],
                            lq[row:row + HD, ih * 512:(ih + 1) * 512],
                            True, True, tile_position=(row, 0))
                    e_t = epool.tile([P, N], BF16, name="e")
                    nc.scalar.activation(out=e_t, in_=sc,
                                         func=mybir.ActivationFunctionType.Exp,
                                         scale=SCALE)
                    av_q.append((n_jt, e_t))
                    if len(av_q) > 1:
                        jt_a, e_a = av_q.pop(0)
                        for ih in range(2):
                            _mm(nc, o_hold[h][ih], v1[jts[jt_a]][:, h, :],
                                e_a[:, ih * 512:(ih + 1) * 512],
                                jt_a == 0, False)
                    if h == 0 and n_jt >= 4:
                        emit_qk23_chunk(n_jt - 4)
                    if h >= 1 and n_jt == 1:
                        # the previous head's normalize, emitted after this
                        # head's first scores so the PE FIFO never stalls on
                        # the o_sb evacuation
                        drain_finish(h - 1)
                jt_a, e_a = av_q.pop(0)
                for ih in range(2):
                    _mm(nc, o_hold[h][ih], v1[jts[jt_a]][:, h, :],
                        e_a[:, ih * 512:(ih + 1) * 512],
                        False, True)

            # ---------------- tail: drain h3 + output projection ----------
            drain_copies(NHC - 1, tail=True)
            warm_chain("tail", 8)   # keep HAM released through the drain
            drain_finish(NHC - 1)
            for it in range(TT):
                # 5 psum buffers in rotation (2 sc-tag + 3 o-tag) so the
                # matmul->evacuate->store pipeline hides semaphore latency
                if it % 2 == 0:
                    yps = ps.tile([P, 512], FP32, name=f"yps{it}", tag="sc",
                                  bufs=2)
                else:
                    yps = ps.tile([P, 512], FP32, name=f"yps{it}", tag="o",
                                  bufs=3)
                _mm(nc, yps, resT[0][:, it * P:(it + 1) * P], wo_sb[0],
                    True, False)
                _mm(nc, yps, resT[1][:, it * P:(it + 1) * P], wo_sb[1],
                    False, True)
                ysb = ypool.tile([P, 512], BF16, name="ysb", bufs=4)
                if it % 2 == 0:
                    nc.vector.tensor_copy(ysb, yps)
                else:
                    nc.scalar.copy(ysb, yps)
                eng = (nc.sync, nc.gpsimd, nc.scalar)[it % 3]
                eng.dma_start(out=y[it * P:(it + 1) * P, :], in_=ysb)
    if compile:
        nc.compile()
        nc.finalize()
    return nc


_CACHE = {}


def _get_program():
    if "nc" not in _CACHE:
        _CACHE["nc"] = build_program()
    return _CACHE["nc"]


def make_in_maps(x, gn_scale, gn_bias, w_qkv, w_out):
    x = np.ascontiguousarray(x, dtype=np.float32)
    w_qkv = np.asarray(w_qkv, dtype=np.float32)
    w_out = np.asarray(w_out, dtype=np.float32)
    gn_scale = np.asarray(gn_scale, dtype=np.float32)
    gn_bias = np.asarray(gn_bias, dtype=np.float32)
    # per-head column blocks of w_qkv: head h -> [q | k | v] at 3*HD*h
    qcols = [w_qkv[:, 3 * HD * h:3 * HD * h + HD] for h in range(N_HEADS)]
    kcols = [w_qkv[:, 3 * HD * h + HD:3 * HD * h + 2 * HD]
             for h in range(N_HEADS)]
    vcols = [w_qkv[:, 3 * HD * h + 2 * HD:3 * HD * h + 3 * HD]
             for h in range(N_HEADS)]
    in_maps = []
    for cid in range(NCORES):
        b, hg = divmod(cid, 2)
        hs = [4 * hg + l for l in range(NHC)]
        xb = x[b].reshape(N, C)
        wqk = np.concatenate(
            [qcols[hs[0]], qcols[hs[1]], kcols[hs[0]], kcols[hs[1]],
             qcols[hs[2]], qcols[hs[3]], kcols[hs[2]], kcols[hs[3]]], axis=1)
        wv = np.concatenate([vcols[h] for h in hs], axis=1)
        wo = np.concatenate([w_out[HD * h:HD * (h + 1), :] for h in hs],
                            axis=0)
        in_maps.append({
            "xT": np.ascontiguousarray(xb.T.astype(ml_dtypes.bfloat16)),
            "wqk": np.ascontiguousarray(wqk.astype(ml_dtypes.bfloat16)),
            "wv": np.ascontiguousarray(wv.astype(ml_dtypes.bfloat16)),
            "wo": np.ascontiguousarray(wo.astype(ml_dtypes.bfloat16)),
            "gsc": gn_scale,
            "gbi": gn_bias,
        })
    return in_maps


def kernel(x, gn_scale, gn_bias, w_qkv, w_out, b_out, _trace=False,
           _trace_kwargs=None):
    x = np.asarray(x, dtype=np.float32)
    b_out = np.asarray(b_out, dtype=np.float32)
    nc = _get_program()
    in_maps = make_in_maps(x, gn_scale, gn_bias, w_qkv, w_out)
    kw = {}
    if _trace:
        kw = dict(trace=True, **(_trace_kwargs or {}))
    res = run_bass_kernel_spmd(nc, in_maps, list(range(NCORES)), **kw)
    _CACHE["last_results"] = res
    out = np.empty((B, N, C), np.float32)
    for b in range(B):
        y0 = res.results[2 * b]["y"].astype(np.float32)
        y1 = res.results[2 * b + 1]["y"].astype(np.float32)
        out[b] = y0 + y1 + x[b].reshape(N, C) + b_out
    return out.reshape(B, HH, WW, C)

